# revision 16
# baseline (speedup 1.0000x reference)
"""Trainium2 Bass kernel for nn_MultiHeadModel (segment_reduce), 8-core SPMD.

Reference math:
    xp  = x @ Wp + bp                              # [N, 256]
    class_emb[g] = (sum_{i in g} m_i * xp_i) / n_g # [G, 256]  (segment mean)
    h   = concat(repeat(class_emb, C), xp[idx])    # [G*C, 512]
    out = relu(relu(h@W1+b1)@W2+b2) @ W3 + b3      # [G*C, 1]
(edge_attr projection in the reference is dead code - output never uses it.)

Structure:
  *  segment mean over xp == (segment sum over raw x) @ Wp (+ n*bp), and the
     projections commute with the concat-split of W1, so with host-folded
     Wt = Wp@W1_top, Wb = Wp@W1_bot the whole left side becomes
       cls1[g]  = (segsum(x)/n) @ Wt + (bp@W1_top + b1)
       h1[r]    = relu(x[idx] @ Wb + bp@W1_bot + cls1[g(r)])
     The [N,256] projection is never materialized.
  *  batch is sorted -> graphs shard contiguously: core k owns graphs
     [128k,128k+128). Host drops masked-out rows (mask==0 contributes
     nothing) and streams x plus a graph-id column, partition-major,
     4 node-tiles per DMA. A one-hot indicator built on-device (iota
     is_equal) turns the segment sum into PE matmuls.
  *  The stream is ordered so graphs [0,64) finish first: their class
     chain + h1/h2/out tail runs DURING the second half of the stream.
  *  x[idx] rows (cross-shard) are host-gathered transposed; their h1
     matmuls are interleaved into the stream to keep the PE warm (HAM).
  *  repeat(class_emb, C): step-0 free-dim AP broadcast.
  *  One packed const DMA carries all weights (f32 views via bitcast).
  *  Matmuls in float32r (fp32_mode=HIGH, ~2 cyc/row, ~1.5e-4 rel err).
     Everything is computed transposed (features on partitions).
"""
import numpy as np
from contextlib import ExitStack

import concourse.bacc as bacc
import concourse.mybir as mybir
from concourse.tile import TileContext
from concourse.bass_utils import run_bass_kernel_spmd

M = 8                 # cores
G = 1024              # graphs
C = 16                # classes
GL = G // M           # graphs per core (128)
D = 256
D2 = 512
ROWS = G * C // M     # MLP rows per core (2048)
NCH = ROWS // 512     # 512-wide row chunks (4)
FW = D + 1            # stream row width: 256 x-feats + 1 graph-id
SUP = 4               # node-tiles per stream DMA

f32 = mybir.dt.float32
f32r = mybir.dt.float32r
Relu = mybir.ActivationFunctionType.Relu
Copy = mybir.ActivationFunctionType.Copy

# ---- packed constant layout (columns of a [128, CW] tile) --------------
_off = {}
_c = 0
def _span(name, w):
    global _c
    _off[name] = (_c, w)
    _c += w
for _k in range(2):
    _span(f"wb{_k}", D2)      # (Wp@W1_bot)  K-chunk k    [128, 512]
for _k in range(2):
    _span(f"xg{_k}", ROWS)
CP1 = _c                      # job-critical prefix (wb + gathered rows)
for _k in range(2):
    _span(f"wt{_k}", D2)      # (Wp@W1_top)  K-chunk k    [128, 512]
for _k in range(4):
    _span(f"w2{_k}", D)
for _k in range(2):
    _span(f"w3{_k}", 1)
for _k in range(4):
    _span(f"cb{_k}", 1)       # bp@(W1_top+W1_bot) + b1, chunk  [128, 1]
for _k in range(2):
    _span(f"b2{_k}", 1)
_span("b3", 1)
_span("invA", 1)              # 1/n for graphs 0..63   (parts 0..63)
_span("invB", 1)              # 1/n for graphs 64..127 (parts 0..63)
_span("ident", 64)
CW = _c

_cache = {}


def _build(NT, BA):
    """NT total node tiles; tiles [0,BA) hold graphs 0..63, rest 64..127."""
    NS = (NT + SUP - 1) // SUP
    SA = (BA - 1) // SUP          # super-tile containing the half boundary
    nc = bacc.Bacc(None, target_bir_lowering=False, debug=False)
    xci = nc.dram_tensor("xci", [128, NT * FW], f32r, kind="ExternalInput")
    iot = nc.dram_tensor("iot", [128, GL], f32r, kind="ExternalInput")
    cpk = nc.dram_tensor("cpk", [128, CW], f32r, kind="ExternalInput")
    out = nc.dram_tensor("out", [1, ROWS], f32, kind="ExternalOutput")

    with TileContext(nc) as tc, ExitStack() as ctx:
        cst = ctx.enter_context(tc.tile_pool(name="cst", bufs=1))
        stream = ctx.enter_context(tc.tile_pool(name="stream", bufs=6))
        pseg = ctx.enter_context(tc.tile_pool(name="pseg", bufs=1, space="PSUM"))
        pmisc = ctx.enter_context(tc.tile_pool(name="pmisc", bufs=2, space="PSUM"))
        pml = ctx.enter_context(tc.tile_pool(name="pml", bufs=4, space="PSUM"))

        iota_t = cst.tile([128, GL], f32r, tag="iota")
        nc.sync.dma_start(out=iota_t[:], in_=iot[:])
        ctile = cst.tile([128, CW], f32r, tag="cpk")
        nc.sync.dma_start(out=ctile[:, :CP1], in_=cpk[:, :CP1])

        def stream_dma(st):
            t0 = st * SUP
            n_sub = min(SUP, NT - t0)
            stile = stream.tile([128, SUP * FW], f32r, tag="s")
            nc.sync.dma_start(out=stile[:, :n_sub * FW],
                              in_=xci[:, t0 * FW:(t0 + n_sub) * FW])
            return stile, n_sub

        NSH = min(2, (NT + SUP - 1) // SUP)
        head = [stream_dma(st) for st in range(NSH)]

        nc.sync.dma_start(out=ctile[:, CP1:], in_=cpk[:, CP1:])

        def cs(name, dt=f32r):
            o, w = _off[name]
            ap = ctile[:, o:o + w]
            return ap.bitcast(dt) if dt is not f32r else ap

        psA = pseg.tile([64, D], f32, tag="psA")
        psB = pseg.tile([64, D], f32, tag="psB")
        h1pre = [[None] * NCH for _ in range(4)]

        def h1pre_group(m1, n):
            ph = pml.tile([128, 512], f32, tag="mlp")
            for k2 in range(2):
                nc.tensor.matmul(out=ph[:],
                                 lhsT=cs(f"wb{k2}")[:, m1 * 128:(m1 + 1) * 128],
                                 rhs=cs(f"xg{k2}")[:, n * 512:(n + 1) * 512],
                                 start=(k2 == 0), stop=(k2 == 1))
            t = cst.tile([128, 512], f32, tag=f"h1p{m1}{n}")
            nc.scalar.activation(out=t[:], in_=ph[:], func=Copy)
            h1pre[m1][n] = t

        def cls_chain(half):
            ps, invn = (psA, "invA") if half == 0 else (psB, "invB")
            sxs = cst.tile([64, D], f32, tag=f"sxs{half}")
            nc.vector.tensor_scalar_mul(out=sxs[:], in0=ps[:],
                                        scalar1=cs(invn, f32)[:64, :1])
            sxT = []
            for c2 in range(2):
                pt = pmisc.tile([128, 64], f32, tag="mm")
                nc.tensor.transpose(out=pt[:], in_=sxs[:, c2 * 128:(c2 + 1) * 128],
                                    identity=cs("ident", f32)[:64, :])
                st_ = cst.tile([128, 64], f32r, tag=f"sxT{half}{c2}")
                nc.vector.tensor_copy(out=st_[:], in_=pt[:])
                sxT.append(st_)
            cls1b = []
            for m1 in range(4):
                p1_ = pmisc.tile([128, 64], f32, tag="mm")
                for k2 in range(2):
                    nc.tensor.matmul(out=p1_[:],
                                     lhsT=cs(f"wt{k2}")[:, m1 * 128:(m1 + 1) * 128],
                                     rhs=sxT[k2][:], start=(k2 == 0), stop=(k2 == 1))
                cb = cst.tile([128, 64], f32, tag=f"cb{half}{m1}")
                nc.vector.tensor_scalar_add(out=cb[:], in0=p1_[:],
                                            scalar1=cs(f"cb{m1}", f32)[:, :1])
                cls1b.append(cb)
            return cls1b

        out_sb = cst.tile([1, ROWS], f32, tag="osb")

        def finish_chunk(n, cls1b):
            gl0 = (n % 2) * 32          # column offset within the half
            h1n = []
            for m1 in range(4):
                hp = h1pre[m1][n]
                nc.vector.tensor_tensor(
                    out=hp[:].rearrange("p (g c) -> p g c", c=C),
                    in0=hp[:].rearrange("p (g c) -> p g c", c=C),
                    in1=cls1b[m1][:, gl0:gl0 + 32, None].to_broadcast([128, 32, C]),
                    op=mybir.AluOpType.add,
                )
                h = cst.tile([128, 512], f32r, tag=f"h1{m1}{n}")
                nc.scalar.activation(out=h[:], in_=hp[:], func=Relu)
                h1n.append(h)
            h2n = []
            for m2 in range(2):
                ph2 = pml.tile([128, 512], f32, tag="mlp")
                for k4 in range(4):
                    nc.tensor.matmul(out=ph2[:],
                                     lhsT=cs(f"w2{k4}")[:, m2 * 128:(m2 + 1) * 128],
                                     rhs=h1n[k4][:],
                                     start=(k4 == 0), stop=(k4 == 3))
                h = cst.tile([128, 512], f32r, tag=f"h2{m2}{n}")
                nc.scalar.activation(out=h[:], in_=ph2[:], func=Relu,
                                     bias=cs(f"b2{m2}", f32)[:, :1])
                h2n.append(h)
            po = pml.tile([1, 512], f32, tag="mlp")
            for k2 in range(2):
                nc.tensor.matmul(out=po[:], lhsT=cs(f"w3{k2}")[:, :1],
                                 rhs=h2n[k2][:], start=(k2 == 0), stop=(k2 == 1))
            nc.vector.tensor_scalar_add(out=out_sb[:1, n * 512:(n + 1) * 512],
                                        in0=po[:], scalar1=cs("b3", f32)[:1, :1])
            nc.sync.dma_start(out=out[:1, n * 512:(n + 1) * 512],
                              in_=out_sb[:1, n * 512:(n + 1) * 512])

        # job pacing: h1pre groups for n in {0,1} early (needed by the
        # half-A tail), n in {2,3} during the B half of the stream.
        jobsA = [(m1, n) for n in range(2) for m1 in range(4)]
        jobsB = [(m1, n) for n in range(2, 4) for m1 in range(4)]
        itA, itB = iter(jobsA), iter(jobsB)
        clsA = [None]
        post = {SA + 1: lambda: finish_chunk(0, clsA[0]),
                SA + 2: lambda: finish_chunk(1, clsA[0])}

        for st in range(NS):
            t0 = st * SUP
            if st < NSH:
                stile, n_sub = head[st]
            else:
                stile, n_sub = stream_dma(st)
            # const-dependent jobs ahead of this super-tile's seg matmuls
            for _ in range(3):
                job = next(itA, None) or next(itB, None)
                if job:
                    h1pre_group(*job)
            for s in range(n_sub):
                t = t0 + s
                ind_t = stream.tile([128, GL], f32r, tag="ind")
                nc.vector.tensor_tensor(
                    out=ind_t[:],
                    in0=stile[:, s * FW + D:s * FW + D + 1].to_broadcast([128, GL]),
                    in1=iota_t[:],
                    op=mybir.AluOpType.is_equal,
                )
                if t < BA:
                    nc.tensor.matmul(out=psA[:], lhsT=ind_t[:, :64],
                                     rhs=stile[:, s * FW:s * FW + D],
                                     start=(t == 0), stop=(t == BA - 1))
                else:
                    nc.tensor.matmul(out=psB[:], lhsT=ind_t[:, 64:],
                                     rhs=stile[:, s * FW:s * FW + D],
                                     start=(t == BA), stop=(t == NT - 1))
                if t == BA - 1:
                    for job in itA:          # any half-A jobs not yet emitted
                        h1pre_group(*job)
                    clsA[0] = cls_chain(0)
            if st in post:
                post[st]()

        for job in itB:
            h1pre_group(*job)
        for n, fn in sorted(post.items()):
            if n > NS - 1:
                fn()
        clsB = cls_chain(1)
        finish_chunk(2, clsB)
        finish_chunk(3, clsB)

    nc.compile()
    return nc


def _pack_consts(Wt, Wb, W2, W3, cbias, b2, b3, invA, invB, xgt):
    cpk = np.zeros((128, CW), np.float32)
    def put(name, arr):
        o, w = _off[name]
        cpk[:arr.shape[0], o:o + w] = arr
    for k in range(2):
        put(f"wt{k}", Wt[k * 128:(k + 1) * 128])
        put(f"wb{k}", Wb[k * 128:(k + 1) * 128])
        put(f"w3{k}", W3[k * 128:(k + 1) * 128])
        put(f"b2{k}", b2[k * 128:(k + 1) * 128, None])
        put(f"xg{k}", xgt[k * 128:(k + 1) * 128])
    for k in range(4):
        put(f"w2{k}", W2[k * 128:(k + 1) * 128])
        put(f"cb{k}", cbias[k * 128:(k + 1) * 128, None])
    cpk[0, _off["b3"][0]] = b3[0]
    put("invA", invA[:, None])
    put("invB", invB[:, None])
    put("ident", np.eye(64, dtype=np.float32))
    return np.ascontiguousarray(cpk)


def kernel(x, edge_attr, batch, target_node_mask, true_nodes_idx,
           Wp, bp, W1, b1, W2, b2, W3, b3,
           num_graphs=G, num_classes=C, **_):
    x = np.ascontiguousarray(np.asarray(x), dtype=np.float32)
    batch = np.asarray(batch).astype(np.int64)
    mask = np.asarray(target_node_mask).astype(bool)
    idx = np.asarray(true_nodes_idx).astype(np.int64)
    Wp = np.asarray(Wp, np.float32)
    W1 = np.asarray(W1, np.float32)
    W2 = np.ascontiguousarray(np.asarray(W2), np.float32)
    W3 = np.ascontiguousarray(np.asarray(W3), np.float32)
    bp = np.asarray(bp, np.float32)
    b1 = np.asarray(b1, np.float32)
    b2 = np.asarray(b2, np.float32)
    b3 = np.asarray(b3, np.float32)

    # constant-fold the initial projection into W1's two halves
    Wt = (Wp @ W1[:D]).astype(np.float32)          # [256, 512]
    Wb = (Wp @ W1[D:]).astype(np.float32)          # [256, 512]
    cbias = (bp @ (W1[:D] + W1[D:]) + b1).astype(np.float32)  # [512]

    ncount = np.bincount(batch[mask], minlength=G).astype(np.float32)
    with np.errstate(divide="ignore"):
        inv_all = (np.float32(1.0) / ncount).astype(np.float32)

    core = batch // GL
    halfA = (batch % GL) < 64
    selA = [np.flatnonzero((core == k) & mask & halfA) for k in range(M)]
    selB = [np.flatnonzero((core == k) & mask & ~halfA) for k in range(M)]
    BA = max(1, max((len(r) + 127) // 128 for r in selA))
    BB = max(1, max((len(r) + 127) // 128 for r in selB))
    NT = BA + BB

    if (NT, BA) not in _cache:
        _cache[(NT, BA)] = _build(NT, BA)
    nc = _cache[(NT, BA)]

    in_maps = []
    iot = np.ascontiguousarray(
        np.broadcast_to(np.arange(GL, dtype=np.float32), (128, GL)))
    for k in range(M):
        xci3 = np.zeros((NT * 128, FW), np.float32)
        xci3[:, D] = -1.0
        for rows, lo in ((selA[k], 0), (selB[k], BA * 128)):
            nk = len(rows)
            xci3[lo:lo + nk, :D] = x[rows]
            xci3[lo:lo + nk, D] = (batch[rows] - k * GL).astype(np.float32)
        xci = np.ascontiguousarray(
            xci3.reshape(NT, 128, FW).transpose(1, 0, 2).reshape(128, NT * FW))
        invA = inv_all[k * GL:k * GL + 64]
        invB = inv_all[k * GL + 64:(k + 1) * GL]
        xgt = np.ascontiguousarray(x[idx[k * ROWS:(k + 1) * ROWS]].T)
        cpk = _pack_consts(Wt, Wb, W2, W3, cbias, b2, b3, invA, invB, xgt)
        in_maps.append(dict(xci=xci, cpk=cpk, iot=iot))

    res = run_bass_kernel_spmd(nc, in_maps, list(range(M)))
    out = np.concatenate([res.results[k]["out"].reshape(ROWS) for k in range(M)])
    return out.reshape(G * C, 1).astype(np.float32)


# revision 18
# speedup vs baseline: 1.0787x; 1.0787x over previous
"""Trainium2 Bass kernel for nn_MultiHeadModel (segment_reduce), 8-core SPMD.

Reference math:
    xp  = x @ Wp + bp                              # [N, 256]
    class_emb[g] = (sum_{i in g} m_i * xp_i) / n_g # [G, 256]  (segment mean)
    h   = concat(repeat(class_emb, C), xp[idx])    # [G*C, 512]
    out = relu(relu(h@W1+b1)@W2+b2) @ W3 + b3      # [G*C, 1]
(edge_attr projection in the reference is dead code - output never uses it.)

Structure:
  *  segment mean over xp == (segment sum over raw x) @ Wp (+ n*bp), and the
     projections commute with the concat-split of W1, so with host-folded
     Wt = Wp@W1_top, Wb = Wp@W1_bot the whole left side becomes
       cls1[g]  = (segsum(x)/n) @ Wt + (bp@W1_top + b1)
       h1[r]    = relu(x[idx] @ Wb + bp@W1_bot + cls1[g(r)])
     The [N,256] projection is never materialized.
  *  batch is sorted -> graphs shard contiguously: core k owns graphs
     [128k,128k+128). Host drops masked-out rows (mask==0 contributes
     nothing) and streams x plus a graph-id column, partition-major,
     4 node-tiles per DMA. A one-hot indicator built on-device (iota
     is_equal) turns the segment sum into PE matmuls.
  *  The stream is ordered so graphs [0,64) finish first: their class
     chain + h1/h2/out tail runs DURING the second half of the stream.
  *  x[idx] rows (cross-shard) are host-gathered transposed; their h1
     matmuls are interleaved into the stream to keep the PE warm (HAM).
  *  repeat(class_emb, C): step-0 free-dim AP broadcast.
  *  One packed const DMA carries all weights (f32 views via bitcast).
  *  Matmuls in float32r (fp32_mode=HIGH, ~2 cyc/row, ~1.5e-4 rel err).
     Everything is computed transposed (features on partitions).
"""
import numpy as np
from contextlib import ExitStack

import concourse.bacc as bacc
import concourse.mybir as mybir
from concourse.tile import TileContext
from concourse.bass_utils import run_bass_kernel_spmd

M = 8                 # cores
G = 1024              # graphs
C = 16                # classes
GL = G // M           # graphs per core (128)
D = 256
D2 = 512
ROWS = G * C // M     # MLP rows per core (2048)
NCH = ROWS // 512     # 512-wide row chunks (4)
FW = D + 1            # stream row width: 256 x-feats + 1 graph-id
SUP = 4               # node-tiles per stream DMA

f32 = mybir.dt.float32
f32r = mybir.dt.float32r
f16 = mybir.dt.float16
Relu = mybir.ActivationFunctionType.Relu
Copy = mybir.ActivationFunctionType.Copy

# ---- packed constant layout (columns of a [128, CW] tile) --------------
_off = {}
_c = 0
def _span(name, w):
    global _c
    _off[name] = (_c, w)
    _c += w
# spans are in fp16 (2-byte) column units; f32/f32r entries use 2 units/elem
for _k in range(2):
    _span(f"wb{_k}", D2)      # fp16 (Wp@W1_bot) K-chunk      [128, 512]
for _k in range(2):
    _span(f"xg{_k}", ROWS)    # fp16 x[idx]^T K-chunk         [128, 2048]
CP1 = _c                      # job-critical prefix (wb + gathered rows)
for _k in range(2):
    _span(f"wt{_k}", D2)      # fp16 (Wp@W1_top) K-chunk      [128, 512]
for _k in range(4):
    _span(f"w2{_k}", D)       # fp16                          [128, 256]
for _k in range(2):
    _span(f"w3{_k}", 2)       # fp16 (padded to even width)   [128, 1]
for _k in range(4):
    _span(f"cb{_k}", 2)       # f32                           [128, 1]
for _k in range(2):
    _span(f"b2{_k}", 2)       # f32
_span("b3", 2)                # f32
_span("invA", 2)              # f32 1/n graphs 0..63   (parts 0..63)
_span("invB", 2)              # f32 1/n graphs 64..127 (parts 0..63)
_span("ident", 128)           # f32 eye(64)
CW = _c

_cache = {}


def _build(NT, BA):
    """NT total node tiles; tiles [0,BA) hold graphs 0..63, rest 64..127."""
    NS = (NT + SUP - 1) // SUP
    SA = (BA - 1) // SUP          # super-tile containing the half boundary
    nc = bacc.Bacc(None, target_bir_lowering=False, debug=False)
    xci = nc.dram_tensor("xci", [128, NT * FW], f16, kind="ExternalInput")
    iot = nc.dram_tensor("iot", [128, GL], f16, kind="ExternalInput")
    cpk = nc.dram_tensor("cpk", [128, CW], f16, kind="ExternalInput")
    out = nc.dram_tensor("out", [1, ROWS], f32, kind="ExternalOutput")

    with TileContext(nc) as tc, ExitStack() as ctx:
        cst = ctx.enter_context(tc.tile_pool(name="cst", bufs=1))
        stream = ctx.enter_context(tc.tile_pool(name="stream", bufs=6))
        pseg = ctx.enter_context(tc.tile_pool(name="pseg", bufs=1, space="PSUM"))
        pmisc = ctx.enter_context(tc.tile_pool(name="pmisc", bufs=2, space="PSUM"))
        pml = ctx.enter_context(tc.tile_pool(name="pml", bufs=4, space="PSUM"))

        iota_t = cst.tile([128, GL], f16, tag="iota")
        nc.sync.dma_start(out=iota_t[:], in_=iot[:])
        ctile = cst.tile([128, CW], f16, tag="cpk")
        nc.sync.dma_start(out=ctile[:, :CP1], in_=cpk[:, :CP1])

        def stream_dma(st):
            t0 = st * SUP
            n_sub = min(SUP, NT - t0)
            stile = stream.tile([128, SUP * FW], f16, tag="s")
            nc.sync.dma_start(out=stile[:, :n_sub * FW],
                              in_=xci[:, t0 * FW:(t0 + n_sub) * FW])
            return stile, n_sub

        NSH = min(2, (NT + SUP - 1) // SUP)
        head = [stream_dma(st) for st in range(NSH)]

        nc.sync.dma_start(out=ctile[:, CP1:], in_=cpk[:, CP1:])

        def cs(name, dt=f16):
            o, w = _off[name]
            ap = ctile[:, o:o + w]
            return ap.bitcast(dt) if dt is not f16 else ap

        psA = pseg.tile([64, D], f32, tag="psA")
        psB = pseg.tile([64, D], f32, tag="psB")
        h1pre = [[None] * NCH for _ in range(4)]

        def h1pre_group(m1, n):
            ph = pml.tile([128, 512], f32, tag="mlp")
            for k2 in range(2):
                nc.tensor.matmul(out=ph[:],
                                 lhsT=cs(f"wb{k2}")[:, m1 * 128:(m1 + 1) * 128],
                                 rhs=cs(f"xg{k2}")[:, n * 512:(n + 1) * 512],
                                 start=(k2 == 0), stop=(k2 == 1))
            t = cst.tile([128, 512], f32, tag=f"h1p{m1}{n}")
            nc.scalar.activation(out=t[:], in_=ph[:], func=Copy)
            h1pre[m1][n] = t

        def cls_chain(half):
            ps, invn = (psA, "invA") if half == 0 else (psB, "invB")
            sxs = cst.tile([64, D], f32, tag=f"sxs{half}")
            nc.vector.tensor_scalar_mul(out=sxs[:], in0=ps[:],
                                        scalar1=cs(invn, f32)[:64, :1])
            sxT = []
            for c2 in range(2):
                pt = pmisc.tile([128, 64], f32, tag="mm")
                nc.tensor.transpose(out=pt[:], in_=sxs[:, c2 * 128:(c2 + 1) * 128],
                                    identity=cs("ident", f32)[:64, :])
                st_ = cst.tile([128, 64], f16, tag=f"sxT{half}{c2}")
                nc.vector.tensor_copy(out=st_[:], in_=pt[:])
                sxT.append(st_)
            cls1b = []
            for m1 in range(4):
                p1_ = pmisc.tile([128, 64], f32, tag="mm")
                for k2 in range(2):
                    nc.tensor.matmul(out=p1_[:],
                                     lhsT=cs(f"wt{k2}")[:, m1 * 128:(m1 + 1) * 128],
                                     rhs=sxT[k2][:], start=(k2 == 0), stop=(k2 == 1))
                cb = cst.tile([128, 64], f32, tag=f"cb{half}{m1}")
                nc.vector.tensor_scalar_add(out=cb[:], in0=p1_[:],
                                            scalar1=cs(f"cb{m1}", f32)[:, :1])
                cls1b.append(cb)
            return cls1b

        out_sb = cst.tile([1, ROWS], f32, tag="osb")

        def finish_chunk(n, cls1b):
            gl0 = (n % 2) * 32          # column offset within the half
            h1n = []
            for m1 in range(4):
                hp = h1pre[m1][n]
                nc.vector.tensor_tensor(
                    out=hp[:].rearrange("p (g c) -> p g c", c=C),
                    in0=hp[:].rearrange("p (g c) -> p g c", c=C),
                    in1=cls1b[m1][:, gl0:gl0 + 32, None].to_broadcast([128, 32, C]),
                    op=mybir.AluOpType.add,
                )
                h = cst.tile([128, 512], f16, tag=f"h1{m1}{n}")
                nc.scalar.activation(out=h[:], in_=hp[:], func=Relu)
                h1n.append(h)
            h2n = []
            for m2 in range(2):
                ph2 = pml.tile([128, 512], f32, tag="mlp")
                for k4 in range(4):
                    nc.tensor.matmul(out=ph2[:],
                                     lhsT=cs(f"w2{k4}")[:, m2 * 128:(m2 + 1) * 128],
                                     rhs=h1n[k4][:],
                                     start=(k4 == 0), stop=(k4 == 3))
                h = cst.tile([128, 512], f16, tag=f"h2{m2}{n}")
                nc.vector.tensor_scalar(out=h[:], in0=ph2[:],
                                        scalar1=cs(f"b2{m2}", f32)[:, :1],
                                        scalar2=0.0,
                                        op0=mybir.AluOpType.add,
                                        op1=mybir.AluOpType.max)
                h2n.append(h)
            po = pml.tile([1, 512], f32, tag="mlp")
            for k2 in range(2):
                nc.tensor.matmul(out=po[:], lhsT=cs(f"w3{k2}")[:, :1],
                                 rhs=h2n[k2][:], start=(k2 == 0), stop=(k2 == 1))
            nc.vector.tensor_scalar_add(out=out_sb[:1, n * 512:(n + 1) * 512],
                                        in0=po[:], scalar1=cs("b3", f32)[:1, :1])
            nc.sync.dma_start(out=out[:1, n * 512:(n + 1) * 512],
                              in_=out_sb[:1, n * 512:(n + 1) * 512])

        # job pacing: h1pre groups for n in {0,1} early (needed by the
        # half-A tail), n in {2,3} during the B half of the stream.
        jobsA = [(m1, n) for n in range(2) for m1 in range(4)]
        jobsB = [(m1, n) for n in range(2, 4) for m1 in range(4)]
        itA, itB = iter(jobsA), iter(jobsB)
        clsA = [None]
        post = {SA + 1: lambda: finish_chunk(0, clsA[0]),
                SA + 2: lambda: finish_chunk(1, clsA[0])}

        for st in range(NS):
            t0 = st * SUP
            if st < NSH:
                stile, n_sub = head[st]
            else:
                stile, n_sub = stream_dma(st)
            # const-dependent jobs ahead of this super-tile's seg matmuls
            for _ in range(3):
                job = next(itA, None) or next(itB, None)
                if job:
                    h1pre_group(*job)
            for s in range(n_sub):
                t = t0 + s
                ind_t = stream.tile([128, GL], f16, tag="ind")
                nc.vector.tensor_tensor(
                    out=ind_t[:],
                    in0=stile[:, s * FW + D:s * FW + D + 1].to_broadcast([128, GL]),
                    in1=iota_t[:],
                    op=mybir.AluOpType.is_equal,
                )
                if t < BA:
                    nc.tensor.matmul(out=psA[:], lhsT=ind_t[:, :64],
                                     rhs=stile[:, s * FW:s * FW + D],
                                     start=(t == 0), stop=(t == BA - 1))
                else:
                    nc.tensor.matmul(out=psB[:], lhsT=ind_t[:, 64:],
                                     rhs=stile[:, s * FW:s * FW + D],
                                     start=(t == BA), stop=(t == NT - 1))
                if t == BA - 1:
                    for job in itA:          # any half-A jobs not yet emitted
                        h1pre_group(*job)
                    clsA[0] = cls_chain(0)
            if st in post:
                post[st]()

        for job in itB:
            h1pre_group(*job)
        for n, fn in sorted(post.items()):
            if n > NS - 1:
                fn()
        clsB = cls_chain(1)
        finish_chunk(2, clsB)
        finish_chunk(3, clsB)

    nc.compile()
    return nc


def _pack_consts(Wt, Wb, W2, W3, cbias, b2, b3, invA, invB, xgt):
    cpk = np.zeros((128, CW), np.float16)
    def put16(name, arr):
        o, w = _off[name]
        a = np.ascontiguousarray(arr, dtype=np.float16)
        cpk[:a.shape[0], o:o + a.shape[1]] = a
    def put32(name, arr):
        o, w = _off[name]
        a = np.ascontiguousarray(arr, dtype=np.float32).view(np.float16)
        cpk[:a.shape[0], o:o + a.shape[1]] = a
    for k in range(2):
        put16(f"wt{k}", Wt[k * 128:(k + 1) * 128])
        put16(f"wb{k}", Wb[k * 128:(k + 1) * 128])
        put16(f"w3{k}", W3[k * 128:(k + 1) * 128])
        put32(f"b2{k}", b2[k * 128:(k + 1) * 128, None])
        put16(f"xg{k}", xgt[k * 128:(k + 1) * 128])
    for k in range(4):
        put16(f"w2{k}", W2[k * 128:(k + 1) * 128])
        put32(f"cb{k}", cbias[k * 128:(k + 1) * 128, None])
    put32("b3", b3[None, :1])
    put32("invA", invA[:, None])
    put32("invB", invB[:, None])
    put32("ident", np.eye(64, dtype=np.float32))
    return np.ascontiguousarray(cpk)


def kernel(x, edge_attr, batch, target_node_mask, true_nodes_idx,
           Wp, bp, W1, b1, W2, b2, W3, b3,
           num_graphs=G, num_classes=C, **_):
    x = np.ascontiguousarray(np.asarray(x), dtype=np.float32)
    batch = np.asarray(batch).astype(np.int64)
    mask = np.asarray(target_node_mask).astype(bool)
    idx = np.asarray(true_nodes_idx).astype(np.int64)
    Wp = np.asarray(Wp, np.float32)
    W1 = np.asarray(W1, np.float32)
    W2 = np.ascontiguousarray(np.asarray(W2), np.float32)
    W3 = np.ascontiguousarray(np.asarray(W3), np.float32)
    bp = np.asarray(bp, np.float32)
    b1 = np.asarray(b1, np.float32)
    b2 = np.asarray(b2, np.float32)
    b3 = np.asarray(b3, np.float32)

    # constant-fold the initial projection into W1's two halves
    Wt = (Wp @ W1[:D]).astype(np.float32)          # [256, 512]
    Wb = (Wp @ W1[D:]).astype(np.float32)          # [256, 512]
    cbias = (bp @ (W1[:D] + W1[D:]) + b1).astype(np.float32)  # [512]

    ncount = np.bincount(batch[mask], minlength=G).astype(np.float32)
    with np.errstate(divide="ignore"):
        inv_all = (np.float32(1.0) / ncount).astype(np.float32)

    core = batch // GL
    halfA = (batch % GL) < 64
    selA = [np.flatnonzero((core == k) & mask & halfA) for k in range(M)]
    selB = [np.flatnonzero((core == k) & mask & ~halfA) for k in range(M)]
    BA = max(1, max((len(r) + 127) // 128 for r in selA))
    BB = max(1, max((len(r) + 127) // 128 for r in selB))
    NT = BA + BB

    if (NT, BA) not in _cache:
        _cache[(NT, BA)] = _build(NT, BA)
    nc = _cache[(NT, BA)]

    in_maps = []
    iot = np.ascontiguousarray(
        np.broadcast_to(np.arange(GL, dtype=np.float16), (128, GL)))
    for k in range(M):
        xci3 = np.zeros((NT * 128, FW), np.float16)
        xci3[:, D] = -1.0
        for rows, lo in ((selA[k], 0), (selB[k], BA * 128)):
            nk = len(rows)
            xci3[lo:lo + nk, :D] = x[rows].astype(np.float16)
            xci3[lo:lo + nk, D] = (batch[rows] - k * GL).astype(np.float16)
        xci = np.ascontiguousarray(
            xci3.reshape(NT, 128, FW).transpose(1, 0, 2).reshape(128, NT * FW))
        invA = inv_all[k * GL:k * GL + 64]
        invB = inv_all[k * GL + 64:(k + 1) * GL]
        xgt = np.ascontiguousarray(x[idx[k * ROWS:(k + 1) * ROWS]].T)
        cpk = _pack_consts(Wt, Wb, W2, W3, cbias, b2, b3, invA, invB, xgt)
        in_maps.append(dict(xci=xci, cpk=cpk, iot=iot))

    res = run_bass_kernel_spmd(nc, in_maps, list(range(M)))
    out = np.concatenate([res.results[k]["out"].reshape(ROWS) for k in range(M)])
    return out.reshape(G * C, 1).astype(np.float32)


# revision 19
# speedup vs baseline: 1.1118x; 1.0306x over previous
"""Trainium2 Bass kernel for nn_MultiHeadModel (segment_reduce), 8-core SPMD.

Reference math:
    xp  = x @ Wp + bp                              # [N, 256]
    class_emb[g] = (sum_{i in g} m_i * xp_i) / n_g # [G, 256]  (segment mean)
    h   = concat(repeat(class_emb, C), xp[idx])    # [G*C, 512]
    out = relu(relu(h@W1+b1)@W2+b2) @ W3 + b3      # [G*C, 1]
(edge_attr projection in the reference is dead code - output never uses it.)

Structure:
  *  segment mean over xp == (segment sum over raw x) @ Wp (+ n*bp), and the
     projections commute with the concat-split of W1, so with host-folded
     Wt = Wp@W1_top, Wb = Wp@W1_bot the whole left side becomes
       cls1[g]  = (segsum(x)/n) @ Wt + (bp@W1_top + b1)
       h1[r]    = relu(x[idx] @ Wb + bp@W1_bot + cls1[g(r)])
     The [N,256] projection is never materialized.
  *  batch is sorted -> graphs shard contiguously: core k owns graphs
     [128k,128k+128). Host drops masked-out rows (mask==0 contributes
     nothing) and streams x plus a graph-id column, partition-major,
     4 node-tiles per DMA. A one-hot indicator built on-device (iota
     is_equal) turns the segment sum into PE matmuls.
  *  The stream is ordered so graphs [0,64) finish first: their class
     chain + h1/h2/out tail runs DURING the second half of the stream.
  *  x[idx] rows (cross-shard) are host-gathered transposed; their h1
     matmuls are interleaved into the stream to keep the PE warm (HAM).
  *  repeat(class_emb, C): step-0 free-dim AP broadcast.
  *  One packed const DMA carries all weights (f32 views via bitcast).
  *  Matmuls in float32r (fp32_mode=HIGH, ~2 cyc/row, ~1.5e-4 rel err).
     Everything is computed transposed (features on partitions).
"""
import numpy as np
from contextlib import ExitStack

import concourse.bacc as bacc
import concourse.mybir as mybir
from concourse.tile import TileContext
from concourse.bass_utils import run_bass_kernel_spmd

M = 8                 # cores
G = 1024              # graphs
C = 16                # classes
GL = G // M           # graphs per core (128)
D = 256
D2 = 512
ROWS = G * C // M     # MLP rows per core (2048)
NCH = ROWS // 512     # 512-wide row chunks (4)
FW = D + 1            # stream row width: 256 x-feats + 1 graph-id
SUP = 4               # node-tiles per stream DMA

f32 = mybir.dt.float32
f32r = mybir.dt.float32r
f16 = mybir.dt.float16
Relu = mybir.ActivationFunctionType.Relu
Copy = mybir.ActivationFunctionType.Copy

# ---- packed constant layout (columns of a [128, CW] tile) --------------
_off = {}
_c = 0
def _span(name, w):
    global _c
    _off[name] = (_c, w)
    _c += w
# spans are in fp16 (2-byte) column units; f32/f32r entries use 2 units/elem
for _k in range(2):
    _span(f"wb{_k}", D2)      # fp16 (Wp@W1_bot) K-chunk      [128, 512]
for _k in range(2):
    _span(f"xg{_k}", ROWS)    # fp16 x[idx]^T K-chunk         [128, 2048]
CP1 = _c                      # job-critical prefix (wb + gathered rows)
for _k in range(2):
    _span(f"wt{_k}", D2)      # fp16 (Wp@W1_top) K-chunk      [128, 512]
for _k in range(4):
    _span(f"w2{_k}", D)       # fp16                          [128, 256]
for _k in range(2):
    _span(f"w3{_k}", 2)       # fp16 (padded to even width)   [128, 1]
for _k in range(4):
    _span(f"cb{_k}", 2)       # f32                           [128, 1]
for _k in range(2):
    _span(f"b2{_k}", 2)       # f32
_span("b3", 2)                # f32
_span("invA", 2)              # f32 1/n graphs 0..63   (parts 0..63)
_span("invB", 2)              # f32 1/n graphs 64..127 (parts 0..63)
_span("ident", 128)           # f32 eye(64)
CW = _c

_cache = {}


def _build(NT, BA):
    """NT total node tiles; tiles [0,BA) hold graphs 0..63, rest 64..127."""
    NS = (NT + SUP - 1) // SUP
    SA = (BA - 1) // SUP          # super-tile containing the half boundary
    nc = bacc.Bacc(None, target_bir_lowering=False, debug=False)
    xci = nc.dram_tensor("xci", [128, NT * FW], f16, kind="ExternalInput")
    iot = nc.dram_tensor("iot", [128, GL], f16, kind="ExternalInput")
    cpk = nc.dram_tensor("cpk", [128, CW], f16, kind="ExternalInput")
    out = nc.dram_tensor("out", [1, ROWS], f32, kind="ExternalOutput")

    with TileContext(nc) as tc, ExitStack() as ctx:
        cst = ctx.enter_context(tc.tile_pool(name="cst", bufs=1))
        stream = ctx.enter_context(tc.tile_pool(name="stream", bufs=6))
        pseg = ctx.enter_context(tc.tile_pool(name="pseg", bufs=1, space="PSUM"))
        pmisc = ctx.enter_context(tc.tile_pool(name="pmisc", bufs=2, space="PSUM"))
        pml = ctx.enter_context(tc.tile_pool(name="pml", bufs=4, space="PSUM"))

        iota_t = cst.tile([128, GL], f16, tag="iota")
        nc.sync.dma_start(out=iota_t[:], in_=iot[:])

        def stream_dma(st):
            t0 = st * SUP
            n_sub = min(SUP, NT - t0)
            stile = stream.tile([128, SUP * FW], f16, tag="s")
            nc.sync.dma_start(out=stile[:, :n_sub * FW],
                              in_=xci[:, t0 * FW:(t0 + n_sub) * FW])
            return stile, n_sub

        head = [stream_dma(0)]
        ctile = cst.tile([128, CW], f16, tag="cpk")
        nc.sync.dma_start(out=ctile[:, :CP1], in_=cpk[:, :CP1])
        head.append(stream_dma(1))
        nc.sync.dma_start(out=ctile[:, CP1:], in_=cpk[:, CP1:])
        NSH = 2

        def cs(name, dt=f16):
            o, w = _off[name]
            ap = ctile[:, o:o + w]
            return ap.bitcast(dt) if dt is not f16 else ap

        psA = pseg.tile([64, D], f32, tag="psA")
        psB = pseg.tile([64, D], f32, tag="psB")
        h1pre = [[None] * NCH for _ in range(4)]

        def h1pre_group(m1, n):
            ph = pml.tile([128, 512], f32, tag="mlp")
            for k2 in range(2):
                nc.tensor.matmul(out=ph[:],
                                 lhsT=cs(f"wb{k2}")[:, m1 * 128:(m1 + 1) * 128],
                                 rhs=cs(f"xg{k2}")[:, n * 512:(n + 1) * 512],
                                 start=(k2 == 0), stop=(k2 == 1))
            t = cst.tile([128, 512], f32, tag=f"h1p{m1}{n}")
            nc.scalar.activation(out=t[:], in_=ph[:], func=Copy)
            h1pre[m1][n] = t

        def cls_chain(half):
            ps, invn = (psA, "invA") if half == 0 else (psB, "invB")
            sxs = cst.tile([64, D], f32, tag=f"sxs{half}")
            nc.vector.tensor_scalar_mul(out=sxs[:], in0=ps[:],
                                        scalar1=cs(invn, f32)[:64, :1])
            sxT = []
            for c2 in range(2):
                pt = pmisc.tile([128, 64], f32, tag="mm")
                nc.tensor.transpose(out=pt[:], in_=sxs[:, c2 * 128:(c2 + 1) * 128],
                                    identity=cs("ident", f32)[:64, :])
                st_ = cst.tile([128, 64], f16, tag=f"sxT{half}{c2}")
                nc.vector.tensor_copy(out=st_[:], in_=pt[:])
                sxT.append(st_)
            cls1b = []
            for m1 in range(4):
                p1_ = pmisc.tile([128, 64], f32, tag="mm")
                for k2 in range(2):
                    nc.tensor.matmul(out=p1_[:],
                                     lhsT=cs(f"wt{k2}")[:, m1 * 128:(m1 + 1) * 128],
                                     rhs=sxT[k2][:], start=(k2 == 0), stop=(k2 == 1))
                cb = cst.tile([128, 64], f32, tag=f"cb{half}{m1}")
                nc.vector.tensor_scalar_add(out=cb[:], in0=p1_[:],
                                            scalar1=cs(f"cb{m1}", f32)[:, :1])
                cls1b.append(cb)
            return cls1b

        out_sb = cst.tile([1, ROWS], f32, tag="osb")

        def finish_chunk(n, cls1b):
            gl0 = (n % 2) * 32          # column offset within the half
            h1n = []
            for m1 in range(4):
                hp = h1pre[m1][n]
                nc.vector.tensor_tensor(
                    out=hp[:].rearrange("p (g c) -> p g c", c=C),
                    in0=hp[:].rearrange("p (g c) -> p g c", c=C),
                    in1=cls1b[m1][:, gl0:gl0 + 32, None].to_broadcast([128, 32, C]),
                    op=mybir.AluOpType.add,
                )
                h = cst.tile([128, 512], f16, tag=f"h1{m1}{n}")
                nc.scalar.activation(out=h[:], in_=hp[:], func=Relu)
                h1n.append(h)
            h2n = []
            for m2 in range(2):
                ph2 = pml.tile([128, 512], f32, tag="mlp")
                for k4 in range(4):
                    nc.tensor.matmul(out=ph2[:],
                                     lhsT=cs(f"w2{k4}")[:, m2 * 128:(m2 + 1) * 128],
                                     rhs=h1n[k4][:],
                                     start=(k4 == 0), stop=(k4 == 3))
                h = cst.tile([128, 512], f16, tag=f"h2{m2}{n}")
                nc.vector.tensor_scalar(out=h[:], in0=ph2[:],
                                        scalar1=cs(f"b2{m2}", f32)[:, :1],
                                        scalar2=0.0,
                                        op0=mybir.AluOpType.add,
                                        op1=mybir.AluOpType.max)
                h2n.append(h)
            po = pml.tile([1, 512], f32, tag="mlp")
            for k2 in range(2):
                nc.tensor.matmul(out=po[:], lhsT=cs(f"w3{k2}")[:, :1],
                                 rhs=h2n[k2][:], start=(k2 == 0), stop=(k2 == 1))
            nc.vector.tensor_scalar_add(out=out_sb[:1, n * 512:(n + 1) * 512],
                                        in0=po[:], scalar1=cs("b3", f32)[:1, :1])
            nc.sync.dma_start(out=out[:1, n * 512:(n + 1) * 512],
                              in_=out_sb[:1, n * 512:(n + 1) * 512])

        # job pacing: h1pre groups for n in {0,1} early (needed by the
        # half-A tail), n in {2,3} during the B half of the stream.
        jobsA = [(m1, n) for n in range(2) for m1 in range(4)]
        jobsB = [(m1, n) for n in range(2, 4) for m1 in range(4)]
        itA, itB = iter(jobsA), iter(jobsB)
        clsA = [None]
        post = {SA + 1: lambda: finish_chunk(0, clsA[0]),
                SA + 2: lambda: finish_chunk(1, clsA[0])}

        for st in range(NS):
            t0 = st * SUP
            if st < NSH:
                stile, n_sub = head[st]
            else:
                stile, n_sub = stream_dma(st)
            # const-dependent jobs ahead of this super-tile's seg matmuls
            for _ in range(3):
                job = next(itA, None) or next(itB, None)
                if job:
                    h1pre_group(*job)
            # indicator for the whole super-tile, half-width, one DVE op
            # per contiguous same-half run
            ind4 = stream.tile([128, SUP * 64], f16, tag="ind")
            runs = []
            lo = 0
            for s in range(n_sub):
                half = 0 if (t0 + s) < BA else 1
                if runs and runs[-1][2] == half:
                    runs[-1] = (runs[-1][0], runs[-1][1] + 1, half)
                else:
                    runs.append((s, 1, half))
            for (s0_, cnt, half) in runs:
                gid = stile[:, s0_ * FW + D::FW][:, :cnt, None]
                nc.vector.tensor_tensor(
                    out=ind4[:, s0_ * 64:(s0_ + cnt) * 64]
                        .rearrange("p (s g) -> p s g", g=64),
                    in0=gid.to_broadcast([128, cnt, 64]),
                    in1=iota_t[:, None, half * 64:half * 64 + 64]
                        .to_broadcast([128, cnt, 64]),
                    op=mybir.AluOpType.is_equal,
                )
            for s in range(n_sub):
                t = t0 + s
                if t < BA:
                    nc.tensor.matmul(out=psA[:], lhsT=ind4[:, s * 64:(s + 1) * 64],
                                     rhs=stile[:, s * FW:s * FW + D],
                                     start=(t == 0), stop=(t == BA - 1))
                else:
                    nc.tensor.matmul(out=psB[:], lhsT=ind4[:, s * 64:(s + 1) * 64],
                                     rhs=stile[:, s * FW:s * FW + D],
                                     start=(t == BA), stop=(t == NT - 1))
                if t == BA - 1:
                    for job in itA:          # any half-A jobs not yet emitted
                        h1pre_group(*job)
                    clsA[0] = cls_chain(0)
            if st in post:
                post[st]()

        for job in itB:
            h1pre_group(*job)
        for n, fn in sorted(post.items()):
            if n > NS - 1:
                fn()
        clsB = cls_chain(1)
        finish_chunk(2, clsB)
        finish_chunk(3, clsB)

    nc.compile()
    return nc


def _pack_consts(Wt, Wb, W2, W3, cbias, b2, b3, invA, invB, xgt):
    cpk = np.zeros((128, CW), np.float16)
    def put16(name, arr):
        o, w = _off[name]
        a = np.ascontiguousarray(arr, dtype=np.float16)
        cpk[:a.shape[0], o:o + a.shape[1]] = a
    def put32(name, arr):
        o, w = _off[name]
        a = np.ascontiguousarray(arr, dtype=np.float32).view(np.float16)
        cpk[:a.shape[0], o:o + a.shape[1]] = a
    for k in range(2):
        put16(f"wt{k}", Wt[k * 128:(k + 1) * 128])
        put16(f"wb{k}", Wb[k * 128:(k + 1) * 128])
        put16(f"w3{k}", W3[k * 128:(k + 1) * 128])
        put32(f"b2{k}", b2[k * 128:(k + 1) * 128, None])
        put16(f"xg{k}", xgt[k * 128:(k + 1) * 128])
    for k in range(4):
        put16(f"w2{k}", W2[k * 128:(k + 1) * 128])
        put32(f"cb{k}", cbias[k * 128:(k + 1) * 128, None])
    put32("b3", b3[None, :1])
    put32("invA", invA[:, None])
    put32("invB", invB[:, None])
    put32("ident", np.eye(64, dtype=np.float32))
    return np.ascontiguousarray(cpk)


def kernel(x, edge_attr, batch, target_node_mask, true_nodes_idx,
           Wp, bp, W1, b1, W2, b2, W3, b3,
           num_graphs=G, num_classes=C, **_):
    x = np.ascontiguousarray(np.asarray(x), dtype=np.float32)
    batch = np.asarray(batch).astype(np.int64)
    mask = np.asarray(target_node_mask).astype(bool)
    idx = np.asarray(true_nodes_idx).astype(np.int64)
    Wp = np.asarray(Wp, np.float32)
    W1 = np.asarray(W1, np.float32)
    W2 = np.ascontiguousarray(np.asarray(W2), np.float32)
    W3 = np.ascontiguousarray(np.asarray(W3), np.float32)
    bp = np.asarray(bp, np.float32)
    b1 = np.asarray(b1, np.float32)
    b2 = np.asarray(b2, np.float32)
    b3 = np.asarray(b3, np.float32)

    # constant-fold the initial projection into W1's two halves
    Wt = (Wp @ W1[:D]).astype(np.float32)          # [256, 512]
    Wb = (Wp @ W1[D:]).astype(np.float32)          # [256, 512]
    cbias = (bp @ (W1[:D] + W1[D:]) + b1).astype(np.float32)  # [512]

    ncount = np.bincount(batch[mask], minlength=G).astype(np.float32)
    with np.errstate(divide="ignore"):
        inv_all = (np.float32(1.0) / ncount).astype(np.float32)

    core = batch // GL
    halfA = (batch % GL) < 64
    selA = [np.flatnonzero((core == k) & mask & halfA) for k in range(M)]
    selB = [np.flatnonzero((core == k) & mask & ~halfA) for k in range(M)]
    BA = max(1, max((len(r) + 127) // 128 for r in selA))
    BB = max(1, max((len(r) + 127) // 128 for r in selB))
    NT = BA + BB

    if (NT, BA) not in _cache:
        _cache[(NT, BA)] = _build(NT, BA)
    nc = _cache[(NT, BA)]

    in_maps = []
    iot = np.ascontiguousarray(
        np.broadcast_to(np.arange(GL, dtype=np.float16), (128, GL)))
    for k in range(M):
        xci3 = np.zeros((NT * 128, FW), np.float16)
        xci3[:, D] = -1.0
        for rows, lo in ((selA[k], 0), (selB[k], BA * 128)):
            nk = len(rows)
            xci3[lo:lo + nk, :D] = x[rows].astype(np.float16)
            xci3[lo:lo + nk, D] = (batch[rows] - k * GL).astype(np.float16)
        xci = np.ascontiguousarray(
            xci3.reshape(NT, 128, FW).transpose(1, 0, 2).reshape(128, NT * FW))
        invA = inv_all[k * GL:k * GL + 64]
        invB = inv_all[k * GL + 64:(k + 1) * GL]
        xgt = np.ascontiguousarray(x[idx[k * ROWS:(k + 1) * ROWS]].T)
        cpk = _pack_consts(Wt, Wb, W2, W3, cbias, b2, b3, invA, invB, xgt)
        in_maps.append(dict(xci=xci, cpk=cpk, iot=iot))

    res = run_bass_kernel_spmd(nc, in_maps, list(range(M)))
    out = np.concatenate([res.results[k]["out"].reshape(ROWS) for k in range(M)])
    return out.reshape(G * C, 1).astype(np.float32)
